# revision 1
# baseline (speedup 1.0000x reference)
"""Crank-Nicolson PDE solver (batched Thomas) as a Bass/Tile kernel for 8 trn2 cores.

Strategy (pure batch parallelism, 16 systems/core):
  The Thomas solve T_j x = R_j V splits into
    (a) a matrix-only reciprocal recurrence over the spatial axis i --
        precomputed for ALL 256 time steps at once (4096 independent lanes
        per core packed as [128 partitions x 32 free]), and
    (b) per-time-step *linear* first-order recurrences that map 1:1 onto the
        hardware TensorTensorScan instruction (fp32 state):
           dp_i = F1_i * dp_{i-1} + dr_i          (forward scan)
           x_i  = F2_i * x_{i+1}  + dp_i          (reversed scan)
        with dr = F0*V[i+1] + F1*V[i+2] + F2*V[i], where
           F0 = (1+b)*r, F1 = a*r, F2 = c*r, r = 1/denom (Thomas factors).
  Coefficients simplify because S_i/dS == i exactly:
    diff = sig^2 * i^2, drift = (r-q)*i  (independent of S0/K/Smax).
  Host does only O(B*M) prep (terminal payoff, index vectors, sqrt-w folding
  into sigma) and the O(B^2) final linear interpolation.
"""
import os
import sys

for _p in ("/opt/trn_rl_repo", "/root/.axon_site/_ro/trn_rl_repo"):
    if os.path.isdir(_p) and _p not in sys.path:
        sys.path.insert(0, _p)

import numpy as np
import concourse.bass as bass
import concourse.bacc as bacc_mod
import concourse.tile as tile
from concourse import mybir
from concourse.bass_utils import run_bass_kernel_spmd

NCORES = 8
B = 128
M = 512          # spatial intervals -> grid 0..512
N = 256          # time steps
BL = B // NCORES  # 16 batch systems per core
NI = M           # padded spatial rows per system (real rows 0..510, pad 511)
R_RATE, Q_RATE = 0.03, 0.01
CHUNK = 64
NCH = NI // CHUNK  # 8
JL = 32           # j-lanes per partition (256 j / 8 j-groups)
JH = N // JL      # 8
F32 = mybir.dt.float32
AL = mybir.AluOpType
AF = bass.ActivationFunctionType if hasattr(bass, "ActivationFunctionType") else None
if AF is None:
    import bass_rust
    AF = bass_rust.ActivationFunctionType

_CACHED_NC = None
TRACE = False          # set by test.py to capture a profile
LAST_RESULT = None     # BassKernelResults from the most recent run


def build_nc(dt_val: float):
    """Build the per-core Bass program (identical on all cores; SPMD over batch)."""
    CD = 1.0 + 0.5 * dt_val * R_RATE   # diag constant:   D  =  2P + CD
    CB = 1.0 - 0.5 * dt_val * R_RATE   # rhs-diag const:  B1 = -2P + CB

    nc = bacc_mod.Bacc(None, target_bir_lowering=False)
    sigw_d = nc.dram_tensor("sigw", [BL, N, NI], F32, kind="ExternalInput")
    vt_d = nc.dram_tensor("vt", [BL, NI + 2], F32, kind="ExternalInput")
    u_d = nc.dram_tensor("uvec", [128, NI], F32, kind="ExternalInput")
    v0_d = nc.dram_tensor("v0", [BL, NI + 2], F32, kind="ExternalOutput")

    with tile.TileContext(nc) as tc:
        with tc.tile_pool(name="fdram", bufs=1, space="DRAM") as dpool:
            f0_dram = dpool.tile([N, BL, NI], F32, tag="f0d")
            f1_dram = dpool.tile([N, BL, NI], F32, tag="f1d")
            f2_dram = dpool.tile([N, BL, NI], F32, tag="f2d")

            # DRAM views for phase-1 writes, 4-d [b, jh, jl, i] to pair with
            # SBUF tiles whose partition dim splits as (b jh)
            def f_w(fd):
                return fd[:].rearrange("(jh jl) b i -> b jh jl i", jh=JH)

            def sb4(t):  # SBUF [128, jl, i] -> [b, jh, jl, i]
                return t[:].rearrange("(b jh) jl i -> b jh jl i", b=BL)

            sig_r = sigw_d[:].rearrange("b (jh jl) i -> b jh jl i", jh=JH)

            # ---------------- phase 1: factors ----------------
            with (
                tc.tile_pool(name="p1", bufs=2) as p1,
                tc.tile_pool(name="p1tiny", bufs=6) as tiny,
                tc.tile_pool(name="p1const", bufs=1) as cpool,
            ):
                u_sb = cpool.tile([128, NI], F32, tag="u")
                nc.gpsimd.dma_start(u_sb[:], u_d[:])

                c_prev = None
                r_prev = None
                for k in range(NCH):
                    i0 = k * CHUNK
                    sg = p1.tile([128, JL, CHUNK], F32, tag="sig")
                    nc.gpsimd.dma_start(sg[:], sig_r[:, :, :, i0:i0 + CHUNK])

                    P_t = p1.tile([128, JL, CHUNK], F32, tag="P")
                    nc.scalar.activation(P_t[:], sg[:], AF.Square)

                    # materialize the broadcast once (walrus 3-d structs only
                    # fit one sync wait; keep all TT ops on flat 2-d APs)
                    ub3 = (u_sb[:, i0:i0 + CHUNK].unsqueeze(1)
                           .broadcast_to([128, JL, CHUNK]))
                    u_full = p1.tile([128, JL, CHUNK], F32, tag="ufull")
                    nc.scalar.activation(u_full[:], ub3, AF.Copy)
                    a_t = p1.tile([128, JL, CHUNK], F32, tag="a")
                    nc.gpsimd.tensor_tensor(a_t[:], P_t[:], u_full[:], AL.subtract)
                    c_t = p1.tile([128, JL, CHUNK], F32, tag="c")
                    nc.gpsimd.tensor_tensor(c_t[:], P_t[:], u_full[:], AL.add)

                    D_t = p1.tile([128, JL, CHUNK], F32, tag="D")
                    nc.scalar.activation(D_t[:], P_t[:], AF.Copy, bias=CD, scale=2.0)
                    B1_t = p1.tile([128, JL, CHUNK], F32, tag="B1")
                    nc.scalar.activation(B1_t[:], P_t[:], AF.Copy, bias=CB, scale=-2.0)

                    E_t = p1.tile([128, JL, CHUNK], F32, tag="E")
                    nc.gpsimd.tensor_tensor(
                        E_t[:, :, 1:], a_t[:, :, 1:], c_t[:, :, :-1], AL.mult)
                    if k > 0:
                        nc.gpsimd.tensor_tensor(
                            E_t[:, :, 0:1], a_t[:, :, 0:1],
                            c_prev[:, :, CHUNK - 1:CHUNK], AL.mult)

                    r_t = p1.tile([128, JL, CHUNK], F32, tag="r")
                    for ii in range(CHUNK):
                        if k == 0 and ii == 0:
                            nc.vector.reciprocal(r_t[:, :, 0], D_t[:, :, 0])
                            continue
                        rp = (r_t[:, :, ii - 1] if ii > 0
                              else r_prev[:, :, CHUNK - 1])
                        tt = tiny.tile([128, JL], F32, tag="tt")
                        nc.vector.tensor_tensor(tt[:], E_t[:, :, ii], rp, AL.mult)
                        dd = tiny.tile([128, JL], F32, tag="dd")
                        nc.vector.tensor_tensor(dd[:], D_t[:, :, ii], tt[:], AL.subtract)
                        nc.vector.reciprocal(r_t[:, :, ii], dd[:])

                    F0_t = p1.tile([128, JL, CHUNK], F32, tag="F0")
                    nc.vector.tensor_tensor(F0_t[:], B1_t[:], r_t[:], AL.mult)
                    F1_t = p1.tile([128, JL, CHUNK], F32, tag="F1")
                    nc.gpsimd.tensor_tensor(F1_t[:], a_t[:], r_t[:], AL.mult)
                    F2_t = p1.tile([128, JL, CHUNK], F32, tag="F2")
                    nc.gpsimd.tensor_tensor(F2_t[:], c_t[:], r_t[:], AL.mult)
                    if k == NCH - 1:
                        # pad row i=511: identity row so scans pass 0 through
                        nc.vector.memset(F0_t[:, :, CHUNK - 1:CHUNK], 0.0)
                        nc.gpsimd.memset(F1_t[:, :, CHUNK - 1:CHUNK], 0.0)
                        nc.gpsimd.memset(F2_t[:, :, CHUNK - 1:CHUNK], 0.0)

                    nc.gpsimd.dma_start(f_w(f0_dram)[:, :, :, i0:i0 + CHUNK], F0_t[:])
                    nc.gpsimd.dma_start(f_w(f1_dram)[:, :, :, i0:i0 + CHUNK], F1_t[:])
                    nc.gpsimd.dma_start(f_w(f2_dram)[:, :, :, i0:i0 + CHUNK], F2_t[:])

                    c_prev = c_t
                    r_prev = r_t

            # ---------------- phase 2: time loop ----------------
            with (
                tc.tile_pool(name="vpool", bufs=1) as vpool,
                tc.tile_pool(name="fj", bufs=4) as fj,
                tc.tile_pool(name="wk", bufs=2) as wk,
            ):
                V = vpool.tile([BL, NI + 2], F32, tag="V")
                nc.gpsimd.dma_start(V[:], vt_d[:])

                for j in range(N - 1, -1, -1):
                    f0j = fj.tile([BL, NI], F32, tag="f0j")
                    nc.gpsimd.dma_start(f0j[:], f0_dram[j])
                    f1j = fj.tile([BL, NI], F32, tag="f1j")
                    nc.gpsimd.dma_start(f1j[:], f1_dram[j])
                    f2j = fj.tile([BL, NI], F32, tag="f2j")
                    nc.gpsimd.dma_start(f2j[:], f2_dram[j])

                    m1 = wk.tile([BL, NI], F32, tag="m1")
                    nc.gpsimd.tensor_tensor(m1[:], f1j[:], V[:, 2:NI + 2], AL.mult)
                    m2 = wk.tile([BL, NI], F32, tag="m2")
                    nc.gpsimd.tensor_tensor(m2[:], f2j[:], V[:, 0:NI], AL.mult)
                    m0 = wk.tile([BL, NI], F32, tag="m0")
                    nc.vector.tensor_tensor(m0[:], f0j[:], V[:, 1:NI + 1], AL.mult)
                    s01 = wk.tile([BL, NI], F32, tag="s01")
                    nc.vector.tensor_tensor(s01[:], m0[:], m1[:], AL.add)
                    dr = wk.tile([BL, NI], F32, tag="dr")
                    nc.vector.tensor_tensor(dr[:], s01[:], m2[:], AL.add)

                    dp = wk.tile([BL, NI], F32, tag="dp")
                    nc.vector.tensor_tensor_scan(
                        dp[:], f1j[:], dr[:], 0.0, AL.mult, AL.add)
                    # reversed scan writes x straight into V[:, 1:NI+1];
                    # pad row writes V[:, NI+1-? ] .. actually x_511 -> V[512]=0
                    nc.vector.tensor_tensor_scan(
                        V[:, NI:0:-1], f2j[:, ::-1], dp[:, ::-1],
                        0.0, AL.mult, AL.add)

                nc.gpsimd.dma_start(v0_d[:], V[:])

    nc.compile()
    return nc


def _host_prep(S0, K, T, sigma):
    f32 = np.float32
    S0 = np.asarray(S0, f32)
    K = np.asarray(K, f32)
    sigma = np.asarray(sigma, f32)
    dt = f32(T) / f32(N)

    Smax = f32(3.0) * np.maximum(S0, K)
    grid = np.arange(M + 1, dtype=f32) / f32(M)
    S = Smax[:, None] * grid[None, :]
    VT = np.maximum(S - K[:, None], f32(0.0))          # [B, 513]
    VTp = np.zeros((B, NI + 2), f32)
    VTp[:, :M + 1] = VT                                 # col 513 stays 0

    i_idx = np.arange(1, M, dtype=f32)                  # 1..511
    w_i = f32(0.25) * dt * i_idx * i_idx
    u_i = (f32(0.25) * dt * f32(R_RATE - Q_RATE) * i_idx).astype(f32)
    u_pad = np.zeros(NI, f32)
    u_pad[:M - 1] = u_i
    u_rep = np.broadcast_to(u_pad, (128, NI)).copy()

    # sigw[b, j, i] = sigma[b, i+1, j] * sqrt(w_i); pad row i=511 -> 1.0
    sig = sigma[:, 1:M, :N]                             # [B, 511, 256]
    sw = np.sqrt(w_i).astype(f32)
    sigw = np.empty((B, N, NI), f32)
    sigw[:, :, :M - 1] = np.transpose(sig, (0, 2, 1)) * sw[None, None, :]
    sigw[:, :, M - 1] = f32(1.0)
    return sigw, VTp, u_rep, float(dt)


def kernel(S0, K, T, sigma, M_in=None, N_in=None, **kw):
    global _CACHED_NC
    if M_in is None:
        M_in = kw.get("M", M)
    if N_in is None:
        N_in = kw.get("N", N)
    assert int(M_in) == M and int(N_in) == N

    sigw, VTp, u_rep, dt = _host_prep(S0, K, T, sigma)

    if _CACHED_NC is None or _CACHED_NC[0] != dt:
        _CACHED_NC = (dt, build_nc(dt))
    nc = _CACHED_NC[1]

    in_maps = []
    for c in range(NCORES):
        sl = slice(c * BL, (c + 1) * BL)
        in_maps.append({
            "sigw": np.ascontiguousarray(sigw[sl]),
            "vt": np.ascontiguousarray(VTp[sl]),
            "uvec": u_rep,
        })
    global LAST_RESULT
    res = run_bass_kernel_spmd(nc, in_maps, list(range(NCORES)), trace=TRACE)
    LAST_RESULT = res
    V0 = np.concatenate([r["v0"][:, :M + 1] for r in res.results], axis=0)

    # host-side final interpolation (matches torch F.interpolate linear align)
    f32 = np.float32
    pos = np.arange(B, dtype=f32) * f32(M / (B - 1))
    lo = np.floor(pos).astype(np.int32)
    hi = np.minimum(lo + 1, M)
    w = (pos - lo.astype(f32)).astype(f32)
    out = V0[:, lo] * (f32(1.0) - w[None, :]) + V0[:, hi] * w[None, :]
    return out.astype(np.float32)


# handle positional-name variants the harness may use
def kernel_entry(**inputs):
    return kernel(inputs["S0"], inputs["K"], inputs["T"], inputs["sigma"],
                  inputs.get("M"), inputs.get("N"))

